# revision 5
# baseline (speedup 1.0000x reference)
"""IsoMaxPlus first-part kernel for TRN2 (8 NeuronCores, data-parallel on B).

out[b, c] = -|s| * sqrt(max(2 - 2 * <f_b/||f_b||, p_c/||p_c||>, 1e-12))

Host prep (per core shard of 8192 rows): features are cast to bf16 and
pre-transposed to d-major layout [128 dpart, 64 blocks, 512] so the device
needs no DMA transposes or casts; prototypes are zero-padded to [1024, 512].

Device per core:
  prolog: 8 big DMAs load all features into SBUF (64KB/partition);
          prototypes are row-normalized (fp32, negated scale, bf16 cast)
          then transposed 128x128-wise on the TensorEngine into
          pnT [128, 4, 1024] bf16.
  main:   8 groups x 8 blocks of 128 rows. Per block: DVE squares the
          bf16 feature slab, PE reduces it against a ones-vector into
          n2 (row norms, exact fp32 psum accumulation), DVE copies n2
          out; per group one ACT Sqrt + DVE reciprocal turn 8 norms into
          the fused scale 2s^2/||f||. Then per block 8 bf16 matmuls
          accumulate dots=-f.p_hat into psum [128,1000], ACT computes
          sqrt(scale*dots + 2s^2) = |s|*dist, GpSimd negates, SP DMAs out.
All matmuls run back-to-back so the PE stays HAM-warm (2.4 GHz).
"""

import numpy as np
from contextlib import ExitStack

import ml_dtypes

import concourse.bass as bass
import concourse.tile as tile
from concourse import bacc, masks, mybir
from concourse.bass import ts
from concourse.bass_utils import run_bass_kernel_spmd

N_CORES = 8
B, D, C = 65536, 512, 1000
CP = 1024                  # prototypes padded (zeros) for 128-alignment
BS = B // N_CORES          # 8192 rows per core
NB = BS // 128             # 64 row blocks
KC = D // 128              # 4 contraction chunks
GB = 8                     # blocks per norm group
NSPLIT = (512, C - 512)    # psum halves (max moving free dim = 512)
CSPLIT = ((0, 256), (256, 256), (512, 256), (768, C - 768))  # fp8 DR chunks
F32 = mybir.dt.float32
BF16 = mybir.dt.bfloat16
F8 = mybir.dt.float8e4
SQRT = mybir.ActivationFunctionType.Sqrt
SQUARE = mybir.ActivationFunctionType.Square


def _emit(nc):
    f_dram = nc.dram_tensor("features", [128, NB, KC, 128], BF16, kind="ExternalInput").ap()
    p_dram = nc.dram_tensor("prototypes", [CP, D], F32, kind="ExternalInput").ap()
    s_dram = nc.dram_tensor("distance_scale", [1], F32, kind="ExternalInput").ap()
    o_dram = nc.dram_tensor("out", [BS, C], F32, kind="ExternalOutput").ap()
    o_pair = o_dram.rearrange("(n j p) c -> p n j c", n=NB // 2, j=2, p=128)

    with tile.TileContext(nc) as tc, ExitStack() as ctx:
        singles = ctx.enter_context(tc.tile_pool(name="singles", bufs=1))

        fT = singles.tile([128, NB, KC, 128], BF16)  # all features, 64KB/part
        pnT = singles.tile([128, KC, CP], BF16)    # -p_hat transposed
        identity = singles.tile([128, 128], BF16)
        ones1 = singles.tile([128, 1], BF16)
        n2a = singles.tile([128, NB], F32)         # row norms^2
        scal = singles.tile([128, NB], F32)        # 2s^2 / ||f||
        s_b = singles.tile([128, 1], F32)
        two_s2 = singles.tile([128, 1], F32)
        inv4s4 = singles.tile([128, 1], F32)

        nc.gpsimd.dma_start(out=s_b[:], in_=s_dram.to_broadcast([128, 1]))
        nc.gpsimd.memset(ones1[:], 1.0)
        masks.make_identity(nc, identity[:])
        s2t = singles.tile([128, 1], F32)
        nc.vector.tensor_mul(s2t[:], s_b[:], s_b[:])
        nc.vector.tensor_scalar_mul(two_s2[:], s2t[:], 2.0)
        q4t = singles.tile([128, 1], F32)
        nc.vector.tensor_mul(q4t[:], two_s2[:], two_s2[:])
        nc.vector.reciprocal(inv4s4[:], q4t[:])

        # ---- prototypes: normalize rows (negated), TensorE-transpose ----
        with tc.tile_pool(name="tpsum", bufs=2, space="PSUM") as tpsum, \
             tc.tile_pool(name="ppool", bufs=2) as ppool, \
             tc.tile_pool(name="psml", bufs=2) as psml:
            for cb in range(CP // 128):
                pt = ppool.tile([128, D], F32, tag="pt")
                nc.sync.dma_start(out=pt[:], in_=p_dram[ts(cb, 128), :])
                pn2 = psml.tile([128, 1], F32, tag="pn2")
                psq = ppool.tile([128, D], F32, tag="psq")
                nc.scalar.activation(psq[:], pt[:], SQUARE, accum_out=pn2[:])
                nc.scalar.sqrt(pn2[:], pn2[:])
                nc.vector.tensor_scalar_max(pn2[:], pn2[:], 1e-12)
                npri = psml.tile([128, 1], F32, tag="npri")
                nc.vector.reciprocal(npri[:], pn2[:])
                nc.vector.tensor_scalar_mul(npri[:], npri[:], -1.0)
                pnb = ppool.tile([128, D], BF16, tag="pnb")
                nc.vector.tensor_scalar_mul(pnb[:], pt[:], npri[:])
                for kc in range(KC):
                    pst = tpsum.tile([128, 128], BF16, tag="pst")
                    nc.tensor.transpose(pst[:], pnb[:, ts(kc, 128)], identity[:])
                    nc.vector.tensor_copy(pnT[:, kc, ts(cb, 128)], pst[:])

        for gi in range(NB // GB):
            nc.sync.dma_start(out=fT[:, ts(gi, GB)], in_=f_dram[:, ts(gi, GB)])

        mpsum = ctx.enter_context(tc.tile_pool(name="mpsum", bufs=3, space="PSUM"))
        npsum = ctx.enter_context(tc.tile_pool(name="npsum", bufs=2, space="PSUM"))
        fsqp = ctx.enter_context(tc.tile_pool(name="fsqp", bufs=10))
        gsml = ctx.enter_context(tc.tile_pool(name="gsml", bufs=2))
        opool = ctx.enter_context(tc.tile_pool(name="opool", bufs=4))

        def emit_norm_phase(g):
            # Row norms for a whole group (emitted one group ahead of its
            # dots phase so neither PE nor ACT hits a group-boundary stall).
            # One ACT op squares the whole group's slab (amortizes the
            # 352-cycle ACT fixed cost); PE reduces per block vs ones.
            fsq = fsqp.tile([128, GB, KC, 128], BF16, tag="fsq")
            nc.scalar.activation(fsq[:], fT[:, ts(g, GB)], SQUARE)
            for j in range(GB):
                ib = g * GB + j
                n2p = npsum.tile([128, 1], F32, tag="n2p")
                for kc in range(KC):
                    nc.tensor.matmul(
                        n2p[:], fsq[:, j, kc, :], ones1[:],
                        start=(kc == 0), stop=(kc == KC - 1),
                        skip_group_check=True,
                    )
                nc.vector.tensor_copy(n2a[:, ib : ib + 1], n2p[:])
            qg = gsml.tile([128, GB], F32, tag="qg")
            nc.scalar.activation(qg[:], n2a[:, ts(g, GB)], SQRT, scale=inv4s4[:])
            nc.vector.reciprocal(scal[:, ts(g, GB)], qg[:])

        def emit_dots_phase(g):
            for j in range(0, GB, 2):
                ot2 = opool.tile([128, 2, C], F32, tag="ot2")
                for h in range(2):
                    ib = g * GB + j + h
                    dots = mpsum.tile([128, C], F32, tag="dots")
                    for kc in range(KC):
                        for lo, width in ((0, NSPLIT[0]), (NSPLIT[0], NSPLIT[1])):
                            nc.tensor.matmul(
                                dots[:, lo : lo + width],
                                fT[:, ib, kc, :],
                                pnT[:, kc, lo : lo + width],
                                start=(kc == 0), stop=(kc == KC - 1),
                                skip_group_check=True,
                            )
                    nc.scalar.activation(
                        ot2[:, h], dots[:], SQRT,
                        bias=two_s2[:], scale=scal[:, ib : ib + 1],
                    )
                # negate + store two blocks at a time (fewer DVE/DMA ops)
                nc.vector.tensor_scalar_mul(ot2[:], ot2[:], -1.0)
                nc.sync.dma_start(out=o_pair[:, (g * GB + j) // 2], in_=ot2[:])

        emit_norm_phase(0)
        for g in range(NB // GB):
            if g + 1 < NB // GB:
                emit_norm_phase(g + 1)
            emit_dots_phase(g)


def build():
    nc = bacc.Bacc("TRN2", target_bir_lowering=False, debug=False,
                   num_devices=N_CORES)
    _emit(nc)
    nc.compile()
    return nc


def _ensure_ntff_hook():
    """Dev-only: restore the axon NTFF profile hook that the trimmed agent
    image's antenv package lacks, so trace=True yields real HW timings."""
    import sys
    import types

    try:
        from antenv.axon_hooks import get_axon_ntff_profile_hook  # noqa: F401
        return
    except ImportError:
        pass
    from trn_agent_boot.trn_boot import _ntff_profile_via_ctypes

    hook = _ntff_profile_via_ctypes("/opt/axon/libaxon_pjrt.so")
    mod = types.ModuleType("antenv.axon_hooks")
    mod.get_axon_ntff_profile_hook = lambda: hook
    mod.set_axon_ntff_profile_hook = lambda h: None
    sys.modules["antenv.axon_hooks"] = mod


def _prep_features(shard):
    x = shard.astype(ml_dtypes.bfloat16)
    return np.ascontiguousarray(x.reshape(NB, 128, KC, 128).transpose(3, 0, 2, 1))


def run(inputs, trace=False):
    if trace:
        _ensure_ntff_hook()
    feats = np.asarray(inputs["features"], dtype=np.float32)
    protos = np.asarray(inputs["prototypes"], dtype=np.float32)
    dscale = np.ascontiguousarray(np.asarray(inputs["distance_scale"], dtype=np.float32))
    protos_p = np.zeros((CP, D), dtype=np.float32)
    protos_p[:C] = protos
    nc = build()
    in_maps = [
        {
            "features": _prep_features(feats[i * BS : (i + 1) * BS]),
            "prototypes": protos_p,
            "distance_scale": dscale,
        }
        for i in range(N_CORES)
    ]
    res = run_bass_kernel_spmd(nc, in_maps, core_ids=list(range(N_CORES)),
                               trace=trace)
    out = np.concatenate([r["out"] for r in res.results], axis=0)
    return out, res


def kernel(**inputs) -> np.ndarray:
    out, _ = run(inputs, trace=False)
    return out
